# revision 14
# baseline (speedup 1.0000x reference)
"""BiLSTM-CRF NLL kernel for 8 TRN2 NeuronCores.

Strategy (data-parallel over batch, per sharding hint):
  - Device (raw Bass, 8 cores, B=32 -> 4 sentences/core): embedding gather
    (indirect DMA from the vocab table, pre-quantized to fp8 e4m3 on host)
    fused with the input projection x @ [W_ih_f | W_ih_b] using fp8
    DoubleRow matmuls (2 contraction rows/cycle, f32 PSUM accumulate).
    The x transpose needed for the matmul contraction is done by the DMA
    XBAR (dma_start_transpose on uint16 views): transposing feature-PAIRS
    keeps 2 fp8 features interleaved per partition, exactly the [two, f]
    free-dim layout DoubleRow consumes. No tensor-engine transposes at all.
  - Host (numpy f32): the small serial tails -- LSTM recurrences over
    T=512, tag projection, CRF forward scan -- which are latency-bound,
    not memory-bound.

Quantization: table*64 -> fp8, W*512 -> fp8, PSUM (exact f32) scaled by
2^-7 on the PSUM->SBUF copy -> fp8 output = 256*xw; host divides by 256.
Validated end to end: final NLL rel err ~4e-5 (gate is 2e-2).

Synchronization: per-engine semaphores; DMA-ish ops increment by 16 at
completion, compute ops by 1. Each instruction may carry several
wait_ge's (each its own queue instruction); transitivity through queue
FIFO order covers the rest.
"""

import sys

for _p in ("/opt/trn_rl_repo",):
    if _p not in sys.path:
        sys.path.insert(0, _p)

import numpy as np
import ml_dtypes

B, T, D_IN, H, V, K = 32, 512, 1824, 512, 50000, 30
START, STOP = K - 2, K - 1
NEG = -10000.0

N_CORES = 8
B_LOC = B // N_CORES          # 4 sentences per core
ROWS = B_LOC * T              # 2048 token rows per core
RT = ROWS // 128              # 16 row tiles of 128
G = 4096                      # 4H * 2 directions

PAIRS = D_IN // 2             # 912 uint16 feature pairs per token
PAD_PAIRS = 1024              # pad to 8 partition tiles of pairs
KT2 = PAD_PAIRS // 128        # 8 contraction tiles (256 features each)
CHW = 256                     # gate-chunk width (DoubleRow moving = 512)
NCH = G // CHW                # 16 gate chunks

SX, SW, SO = 64.0, 512.0, 256.0   # table/weight/output quant scales
COPY_SCALE = SO / (SX * SW)       # 2^-7, exact

_nc_cache = {}


def _build_nc():
    import concourse.bass as bass
    import concourse.mybir as mybir
    from contextlib import ExitStack

    nc = bass.Bass()
    f32 = mybir.dt.float32
    fp8 = mybir.dt.float8e4
    u16 = mybir.dt.uint16

    table = nc.declare_dram_parameter("table", [V, PAIRS], u16, isOutput=False)
    w2 = nc.declare_dram_parameter("w2", [128, NCH, KT2, 2, CHW], fp8,
                                   isOutput=False)
    ids_t = nc.declare_dram_parameter("ids_t", [128, RT], mybir.dt.int32,
                                      isOutput=False)
    xw_out = nc.declare_dram_parameter("xw_out", [ROWS, G], fp8, isOutput=True)

    ctx = ExitStack()
    with ctx:
        # DMA completions racing on one semaphore are unordered, so each
        # wait value must cover *all* increments outstanding at wait time.
        # Per-stream (and per-parity, for double-buffered streams)
        # semaphores keep every wait at such a stable value.
        sem_names = ("ids", "w2g0", "w2g1", "w2g2", "w2g3", "g0", "g1",
                     "tr0", "tr1", "o0", "o1", "pad", "vector", "tensor")
        sems = {e: ctx.enter_context(nc.semaphore(f"s_{e}"))
                for e in sem_names}
        ids_sb = ctx.enter_context(nc.sbuf_tensor([128, RT], mybir.dt.int32))
        w2_sb = ctx.enter_context(
            nc.sbuf_tensor([128, NCH, KT2, 2, CHW], fp8))
        xr = [ctx.enter_context(nc.sbuf_tensor(f"xr{i}", [128, PAD_PAIRS], u16))
              for i in range(2)]
        x2 = [ctx.enter_context(nc.sbuf_tensor(f"x2_{i}", [128, KT2, 128], u16))
              for i in range(2)]
        osb = [ctx.enter_context(nc.sbuf_tensor(f"osb{i}", [128, G], fp8))
               for i in range(2)]
        ps = [ctx.enter_context(nc.psum_tensor(f"ps{i}", [128, CHW], f32))
              for i in range(8)]

        # ops: (queue-engine, fn, semaphore to inc, inc, waits)
        ops = []
        cnt = {e: 0 for e in sem_names}

        def add(engine, sem, inc, fn, waits=()):
            ops.append((engine, fn, sem, inc, list(waits)))
            cnt[sem] += inc
            return cnt[sem]

        # --- setup DMAs ---
        add("sync", "ids", 16,
            lambda: nc.sync.dma_start(out=ids_sb[:], in_=ids_t[:]))
        for c in range(NCH):
            add("sync", f"w2g{c // 4}", 16, lambda c=c: nc.sync.dma_start(
                out=w2_sb[:, c], in_=w2[:, c]))
        # zero the pad pair columns once per gather buffer (never rewritten)
        add("gpsimd", "pad", 1, lambda: nc.gpsimd.memset(xr[0][:, PAIRS:], 0))
        add("gpsimd", "pad", 1, lambda: nc.gpsimd.memset(xr[1][:, PAIRS:], 0))

        def ps_out(o, c):
            return o[:, c * CHW:(c + 1) * CHW]

        for rt in range(RT):
            par = rt % 2
            # gather rt -> xr[par]. WAR vs transpose(rt-2) reading xr is
            # covered by waiting chains(rt-2): chain(rt-2) start implies
            # transpose(rt-2) complete.
            if rt == 0:
                gw = [("ids", 16)]                     # ids loaded
            elif rt == 1:
                gw = []                                # FIFO after gather 0
            else:
                gw = [("tensor", 128 * (rt - 1))]      # chains(rt-2) done
            gp_g = add("gpsimd", f"g{par}", 16,
                       lambda rt=rt: nc.gpsimd.indirect_dma_start(
                           out=xr[rt % 2][:, :PAIRS], out_offset=None,
                           in_=table[:], in_offset=bass.IndirectOffsetOnAxis(
                               ap=ids_sb[:, rt:rt + 1], axis=0)), waits=gw)

            # XBAR transpose: xr [128 tok, 1024 u16] -> x2 [128, 8, 128]
            # x2[p, k, t] = feature-pair (k*128+p) of token t.
            # x2 WAR vs chains(rt-2) covered transitively via the gather.
            tw = [(f"g{par}", gp_g)]
            if rt < 2:
                tw.append(("pad", 2))
            sc_tr = add("scalar", f"tr{par}", 16,
                        lambda rt=rt: nc.scalar.dma_start_transpose(
                            out=x2[rt % 2][:], in_=xr[rt % 2][:]),
                        waits=tw)

            # matmuls: 8 pairs of gate chunks; chunk pair (2p, 2p+1)
            # interleaved k-wise so the x2 stationary tile is reused.
            for p in range(8):
                for k in range(KT2):
                    for ci in range(2):
                        c = 2 * p + ci
                        w = []
                        if k == 0 and ci == 0:
                            if p == 0:
                                w.append((f"tr{par}", sc_tr))   # x2 ready
                            if rt == 0:
                                w.append((f"w2g{(2 * p + 1) // 4}", 64))
                            if p >= 4:
                                # banks reused from pair p-4 of this rt
                                w.append(("vector", 16 * rt + 2 * p - 6))
                            elif rt > 0:
                                # banks reused from pair p+4 of rt-1
                                w.append(("vector", 16 * (rt - 1) + 2 * p + 10))
                        # SwInterleave weights format: per PE column, the 2
                        # fp8 weights (feature pair) adjacent in memory, and
                        # columns (tokens) in REVERSE order -- which is the
                        # XBAR pair-transpose output with ids pre-reversed
                        # per 128-token tile on the host.
                        add("tensor", "tensor", 1, lambda rt=rt, p=p, k=k, c=c:
                            nc.tensor.matmul(
                                ps[c % 8][:],
                                lhsT=x2[rt % 2][:, k, :]
                                .bitcast(mybir.dt.float8e4),
                                rhs=w2_sb[:, c, k, :, :],
                                start=(k == 0), stop=(k == KT2 - 1),
                                perf_mode=mybir.MatmulPerfMode
                                .DoubleRowSwInterleave),
                            waits=w)
                for ci in range(2):
                    c = 2 * p + ci
                    w = [("tensor", 128 * rt + 16 * (p + 1))]
                    if c == 0 and rt >= 2:
                        # osb[par] free: out dma rt-2 (same parity) done
                        w.append((f"o{par}", 16 * (rt // 2)))
                    add("vector", "vector", 1,
                        lambda rt=rt, c=c: nc.vector.tensor_scalar_mul(
                            ps_out(osb[rt % 2], c), ps[c % 8][:], COPY_SCALE),
                        waits=w)

            add("sync", f"o{par}", 16, lambda rt=rt: nc.sync.dma_start(
                out=xw_out[rt * 128:(rt + 1) * 128, :], in_=osb[rt % 2][:]),
                waits=[("vector", 16 * (rt + 1))])

        totals = dict(cnt)
        for engine in ("sync", "gpsimd", "vector", "tensor", "scalar"):
            h = getattr(nc, engine)
            for e, fn, sem, inc, waits in ops:
                if e != engine:
                    continue
                for we, wv in waits:
                    if wv > 0:
                        h.wait_ge(sems[we], wv)
                fn().then_inc(sems[sem], inc)
        for s, tot in totals.items():
            if tot > 0:
                nc.sync.wait_ge(sems[s], tot)

    return nc


def _prep_static(embed_table, W_ih_f, W_ih_b):
    """Host-side quantization + layout (shared across cores)."""
    table8 = (embed_table * SX).astype(ml_dtypes.float8_e4m3fn)
    table_u16 = np.ascontiguousarray(table8.view(np.uint16))  # [V, 912]

    wcat = np.zeros((2 * PAD_PAIRS, G), np.float32)
    wcat[:D_IN, :2048] = W_ih_f
    wcat[:D_IN, 2048:] = W_ih_b
    w8 = (wcat * SW).astype(ml_dtypes.float8_e4m3fn)
    # [f, g] -> [p, c, k, j, g'] with f = 2*(k*128+p)+j, g = c*256+g'
    w2 = np.ascontiguousarray(
        w8.reshape(PAD_PAIRS, 2, NCH, CHW)
        .reshape(KT2, 128, 2, NCH, CHW)
        .transpose(1, 3, 0, 2, 4))
    return table_u16, w2


def _run_device(ids_np, embed_table, W_ih_f, W_ih_b):
    from concourse.bass_utils import run_bass_kernel_spmd

    if "nc" not in _nc_cache:
        _nc_cache["nc"] = _build_nc()
    nc = _nc_cache["nc"]

    table_u16, w2 = _prep_static(embed_table, W_ih_f, W_ih_b)

    in_maps = []
    for c in range(N_CORES):
        ids_loc = ids_np[c * B_LOC:(c + 1) * B_LOC].reshape(ROWS)
        # token order REVERSED within each 128-tile: SwInterleave consumes
        # weight columns last-first, so psum partition m = real token m.
        ids_tr = np.ascontiguousarray(
            ids_loc.reshape(RT, 128)[:, ::-1].T.astype(np.int32))
        in_maps.append({"table": table_u16, "w2": w2, "ids_t": ids_tr})

    res = run_bass_kernel_spmd(nc, in_maps, core_ids=list(range(N_CORES)))
    global _last_exec_ns
    _last_exec_ns = res.exec_time_ns
    xw = np.stack([np.asarray(res.results[c]["xw_out"])
                   .astype(np.float32) for c in range(N_CORES)])
    return xw.reshape(B, T, G) * np.float32(1.0 / SO)


_last_exec_ns = None


def _sigmoid(x):
    return 1.0 / (1.0 + np.exp(-x))


def _lstm(xw, W_hh):
    # xw: [T, B, 4H] f32; returns hs [T, B, H]
    n_b = xw.shape[1]
    h = np.zeros((n_b, H), np.float32)
    c = np.zeros((n_b, H), np.float32)
    hs = np.empty((T, n_b, H), np.float32)
    for t in range(T):
        g = xw[t] + h @ W_hh
        i, f, gg, o = np.split(g, 4, axis=-1)
        c = _sigmoid(f) * c + _sigmoid(i) * np.tanh(gg)
        h = _sigmoid(o) * np.tanh(c)
        hs[t] = h
    return hs


def kernel(ids, tags, embed_table, W_ih_f, W_hh_f, b_f, W_ih_b, W_hh_b,
           b_b, W_tag, b_tag, transitions):
    ids = np.asarray(ids, np.int32)
    tags = np.asarray(tags, np.int32)
    embed_table = np.asarray(embed_table, np.float32)
    W_ih_f = np.asarray(W_ih_f, np.float32)
    W_hh_f = np.asarray(W_hh_f, np.float32)
    b_f = np.asarray(b_f, np.float32)
    W_ih_b = np.asarray(W_ih_b, np.float32)
    W_hh_b = np.asarray(W_hh_b, np.float32)
    b_b = np.asarray(b_b, np.float32)
    W_tag = np.asarray(W_tag, np.float32)
    b_tag = np.asarray(b_tag, np.float32)
    transitions = np.asarray(transitions, np.float32)

    xw = _run_device(ids, embed_table, W_ih_f, W_ih_b)  # [B, T, 4096] f32

    xw_f = np.transpose(xw[:, :, :2048], (1, 0, 2)) + b_f  # [T, B, 2048]
    xw_b = np.transpose(xw[:, :, 2048:], (1, 0, 2)) + b_b

    hf = _lstm(xw_f, W_hh_f)                        # [T, B, H]
    hb = _lstm(xw_b[::-1], W_hh_b)[::-1]

    hcat = np.concatenate([hf, hb], axis=-1)        # [T, B, 2H]
    feats = hcat.reshape(T * B, 2 * H) @ W_tag + b_tag
    feats = np.transpose(feats.reshape(T, B, K), (1, 0, 2))  # [B, T, K]

    # CRF forward (vectorized over batch)
    alpha = np.full((B, K), NEG, np.float32)
    alpha[:, START] = 0.0
    for t in range(T):
        scores = alpha[:, None, :] + transitions[None, :, :] + feats[:, t, :, None]
        m = scores.max(axis=2)
        alpha = m + np.log(np.sum(np.exp(scores - m[:, :, None]), axis=2))
    fin = alpha + transitions[STOP][None, :]
    mf = fin.max(axis=1)
    log_z = mf + np.log(np.sum(np.exp(fin - mf[:, None]), axis=1))

    prev = np.concatenate([np.full((B, 1), START, np.int32), tags], axis=1)
    nxt = np.concatenate([tags, np.full((B, 1), STOP, np.int32)], axis=1)
    gold = transitions[nxt, prev].sum(axis=1)
    gold += np.take_along_axis(
        feats, tags[:, :, None], axis=2
    )[:, :, 0].sum(axis=1)

    return (log_z - gold).astype(np.float32)


# revision 22
# speedup vs baseline: 1.0676x; 1.0676x over previous
"""BiLSTM-CRF NLL kernel for 8 TRN2 NeuronCores.

Strategy (data-parallel over batch, per sharding hint):
  - Device (raw Bass, 8 cores, B=32 -> 4 sentences/core): embedding gather
    (indirect DMA from the vocab table, pre-quantized to fp8 e4m3 on host)
    fused with the input projection x @ [W_ih_f | W_ih_b] using fp8
    DoubleRow matmuls (2 contraction rows/cycle, f32 PSUM accumulate).
    The x transpose needed for the matmul contraction is done by the DMA
    XBAR (dma_start_transpose on uint16 views): transposing feature-PAIRS
    keeps 2 fp8 features interleaved per partition, exactly the [two, f]
    free-dim layout DoubleRow consumes. No tensor-engine transposes at all.
  - Host (numpy f32): the small serial tails -- LSTM recurrences over
    T=512, tag projection, CRF forward scan -- which are latency-bound,
    not memory-bound.

Quantization: table*64 -> fp8, W*512 -> fp8, PSUM (exact f32) scaled by
2^-7 on the PSUM->SBUF copy -> fp8 output = 256*xw; host divides by 256.
Validated end to end: final NLL rel err ~4e-5 (gate is 2e-2).

Synchronization: per-engine semaphores; DMA-ish ops increment by 16 at
completion, compute ops by 1. Each instruction may carry several
wait_ge's (each its own queue instruction); transitivity through queue
FIFO order covers the rest.
"""

import sys

for _p in ("/opt/trn_rl_repo",):
    if _p not in sys.path:
        sys.path.insert(0, _p)

import numpy as np
import ml_dtypes

B, T, D_IN, H, V, K = 32, 512, 1824, 512, 50000, 30
START, STOP = K - 2, K - 1
NEG = -10000.0

N_CORES = 8
B_LOC = B // N_CORES          # 4 sentences per core
ROWS = B_LOC * T              # 2048 token rows per core
RT = ROWS // 128              # 16 row tiles of 128
G = 4096                      # 4H * 2 directions

PAIRS = D_IN // 2             # 912 uint16 feature pairs per token
PAD_PAIRS = 1024              # pad to 8 partition tiles of pairs
KT2 = PAD_PAIRS // 128        # 8 contraction tiles (256 features each)
CHW = 256                     # gate-chunk width (DoubleRow moving = 512)
NCH = G // CHW                # 16 gate chunks

SX, SW, SO = 64.0, 512.0, 256.0   # table/weight/output quant scales
COPY_SCALE = SO / (SX * SW)       # 2^-7, exact

_nc_cache = {}


def _build_nc():
    import concourse.bass as bass
    import concourse.mybir as mybir
    from contextlib import ExitStack

    nc = bass.Bass()
    f32 = mybir.dt.float32
    fp8 = mybir.dt.float8e4
    u16 = mybir.dt.uint16

    table = nc.declare_dram_parameter("table", [V, PAIRS], u16, isOutput=False)
    w2 = nc.declare_dram_parameter("w2", [128, NCH, KT2, 2, CHW], fp8,
                                   isOutput=False)
    ids_t = nc.declare_dram_parameter("ids_t", [128, RT], mybir.dt.int32,
                                      isOutput=False)
    xw_out = nc.declare_dram_parameter("xw_out", [ROWS, G], fp8, isOutput=True)

    NBUF = 8          # gather/transpose pipeline depth (row tiles in flight)

    ctx = ExitStack()
    with ctx:
        # DMA completions racing on one semaphore are unordered, so each
        # wait value must cover *all* increments outstanding at wait time.
        # Per-stream (and per-buffer, for multi-buffered streams)
        # semaphores keep every wait at such a stable value.
        sem_names = (["ids", "pad", "vector", "act", "tensor",
                  "o0", "o1", "o2", "o3"]
                     + [f"wp{p}" for p in range(8)]
                     + [f"g{i}" for i in range(NBUF)]
                     + [f"tr{i}" for i in range(NBUF)])
        sems = {e: ctx.enter_context(nc.semaphore(f"s_{e}"))
                for e in sem_names}
        ids_sb = ctx.enter_context(nc.sbuf_tensor([128, RT], mybir.dt.int32))
        w2_sb = ctx.enter_context(
            nc.sbuf_tensor([128, NCH, KT2, 2, CHW], fp8))
        xr = [ctx.enter_context(nc.sbuf_tensor(f"xr{i}", [128, PAD_PAIRS], u16))
              for i in range(NBUF)]
        x2 = [ctx.enter_context(nc.sbuf_tensor(f"x2_{i}", [128, KT2, 128], u16))
              for i in range(NBUF)]
        osb = [ctx.enter_context(nc.sbuf_tensor(f"osb{i}", [128, G], fp8))
               for i in range(4)]
        ps = [ctx.enter_context(nc.psum_tensor(f"ps{i}", [128, CHW], f32))
              for i in range(8)]

        # ops: (queue-engine, fn, semaphore to inc, inc, waits)
        ops = []
        cnt = {e: 0 for e in sem_names}
        cp = {}           # recorded semaphore checkpoints

        def add(engine, sem, inc, fn, waits=()):
            ops.append((engine, fn, sem, inc, list(waits)))
            cnt[sem] += inc
            return cnt[sem]

        def ps_out(o, c):
            return o[:, c * CHW:(c + 1) * CHW]

        def emit_w2(c, h):
            add("sync", f"wp{c // 2}", 16, lambda c=c, h=h: nc.sync.dma_start(
                out=w2_sb[:, c, 4 * h:4 * h + 4],
                in_=w2[:, c, 4 * h:4 * h + 4]))

        # --- setup DMAs. Order on the sync queue controls the DMA-engine
        # service order: ids, then w2 pair 0 (so chains can start), then the
        # first NBUF gather/XBAR rounds slot in ahead of the rest of the w2
        # stream, which chains then consume at the rate it arrives.
        add("sync", "ids", 16,
            lambda: nc.sync.dma_start(out=ids_sb[:], in_=ids_t[:]))
        for h in range(4):
            emit_w2(h // 2, h % 2)
        # zero the pad pair columns once per gather buffer (never rewritten)
        for i in range(NBUF):
            add("gpsimd", "pad", 1,
                lambda i=i: nc.gpsimd.memset(xr[i][:, PAIRS:], 0))

        def emit_gather_xbar(j):
            emit_gather(j)
            emit_xbar(j)

        def emit_gather(j):
            """Gather for row tile j, software-pipelined NBUF tiles ahead."""
            buf = j % NBUF
            # gather j -> xr[buf]. WAR vs transpose(j-NBUF) reading xr is
            # covered by waiting chains(j-NBUF): their start implies
            # transpose(j-NBUF) complete.
            if j == 0:
                gw = [("ids", 16)]                     # ids loaded
            elif j < NBUF:
                gw = []                                # FIFO after gather 0
            else:
                gw = [(f"tr{j % NBUF}", cp[("tr", j - NBUF)])]
            add("gpsimd", f"g{buf}", 16,
                lambda j=j, buf=buf: nc.gpsimd.indirect_dma_start(
                    out=xr[buf][:, :PAIRS], out_offset=None,
                    in_=table[:], in_offset=bass.IndirectOffsetOnAxis(
                        ap=ids_sb[:, j:j + 1], axis=0)), waits=gw)
            cp[("g", j)] = cnt[f"g{buf}"]

        def emit_xbar(j):
            buf = j % NBUF
            # XBAR transpose: xr [128 tok, 1024 u16] -> x2 [128, 8, 128]
            # x2[p, k, t] = feature-pair (k*128+p) of token t.
            # x2 WAR vs chains(j-NBUF) covered transitively via the gather.
            tw = [(f"g{buf}", cp[("g", j)])]
            if j < NBUF:
                tw.append(("pad", NBUF))
            else:
                tw.append(("tensor", cp[("chain", j - NBUF, 7)]))
            add("sync", f"tr{buf}", 16,
                lambda buf=buf: nc.sync.dma_start_transpose(
                    out=x2[buf][:], in_=xr[buf][:]),
                waits=tw)
            cp[("tr", j)] = cnt[f"tr{buf}"]

        for j in range(NBUF):
            emit_gather(j)
        # sync-queue head order = DMA service order: the first 4 XBARs are
        # front-loaded between w2 halves so the interleaved chain head has
        # its x2 tiles just in time; later XBARs spread into the stream.
        h = 4
        for j in range(NBUF):
            emit_xbar(j)
            for _ in range(3):
                if h < 2 * NCH:
                    emit_w2(h // 2, h % 2)
                    h += 1
        while h < 2 * NCH:
            emit_w2(h // 2, h % 2)
            h += 1

        # Chain jobs (rt, p): p = gate-chunk pair; the first NBUF row tiles
        # are interleaved pair-wise so the PE consumes the streaming w2
        # pairs without idling; later row tiles run sequentially.
        NIL = 4           # row tiles interleaved pair-wise during w2 load
        jobs = [(rt, p) for p in range(8) for rt in range(NIL)]
        jobs += [(rt, p) for rt in range(NIL, RT) for p in range(8)]
        first_copy = {}   # (engine-sem, rt) -> False once emitted
        rt_copy_cnt = {}  # rt -> {sem: cnt} updated at each copy
        done_pairs = {}   # rt -> count of finished pairs

        for i, (rt, p) in enumerate(jobs):
            buf = rt % NBUF
            par = rt % 4
            bank = (2 * i) % 8
            csem = "vector" if i % 2 == 0 else "act"
            for k in range(KT2):
                for ci in range(2):
                    c = 2 * p + ci
                    w = []
                    if k == 0 and ci == 0:
                        if p == 0:
                            w.append((f"tr{buf}", cp[("tr", rt)]))
                        if rt == 0:
                            w.append((f"wp{p}", 64))
                        if i >= 4:
                            w.append(cp[("copyjob", i - 4)])
                    # SwInterleave weights format: per PE column, the 2
                    # fp8 weights (feature pair) adjacent in memory, and
                    # columns (tokens) in REVERSE order -- which is the
                    # XBAR pair-transpose output with ids pre-reversed
                    # per 128-token tile on the host.
                    add("tensor", "tensor", 1, lambda buf=buf, k=k, c=c,
                        bank=bank, ci=ci:
                        nc.tensor.matmul(
                            ps[bank + ci][:],
                            lhsT=x2[buf][:, k, :]
                            .bitcast(mybir.dt.float8e4),
                            rhs=w2_sb[:, c, k, :, :],
                            start=(k == 0), stop=(k == KT2 - 1),
                            perf_mode=mybir.MatmulPerfMode
                            .DoubleRowSwInterleave),
                        waits=w)
            cp[("chain", rt, p)] = cnt["tensor"]
            for ci in range(2):
                c = 2 * p + ci
                w = [("tensor", cp[("chain", rt, p)])]
                if ci == 0 and rt >= 4 and first_copy.setdefault((csem, rt), True):
                    first_copy[(csem, rt)] = False
                    # osb[par] free: out dma rt-4 (same buffer) done
                    w.append((f"o{par}", 16 * (rt // 4)))
                if csem == "vector":
                    add("vector", "vector", 1,
                        lambda rt=rt, c=c, bank=bank, ci=ci:
                        nc.vector.tensor_scalar_mul(
                            ps_out(osb[rt % 4], c), ps[bank + ci][:],
                            COPY_SCALE),
                        waits=w)
                else:
                    add("scalar", "act", 1,
                        lambda rt=rt, c=c, bank=bank, ci=ci:
                        nc.scalar.activation(
                            ps_out(osb[rt % 4], c), ps[bank + ci][:],
                            mybir.ActivationFunctionType.Copy,
                            scale=COPY_SCALE),
                        waits=w)
            cp[("copyjob", i)] = (csem, cnt[csem])
            rt_copy_cnt.setdefault(rt, {})[csem] = cnt[csem]

            done_pairs[rt] = done_pairs.get(rt, 0) + 1
            if done_pairs[rt] == 8:
                add("sync", f"o{par}", 16, lambda rt=rt: nc.sync.dma_start(
                    out=xw_out[rt * 128:(rt + 1) * 128, :],
                    in_=osb[rt % 4][:]),
                    waits=[(sm, cv) for sm, cv in rt_copy_cnt[rt].items()])
                if rt + NBUF < RT:
                    emit_gather_xbar(rt + NBUF)

        totals = dict(cnt)
        for engine in ("sync", "gpsimd", "vector", "tensor", "scalar"):
            h = getattr(nc, engine)
            for e, fn, sem, inc, waits in ops:
                if e != engine:
                    continue
                for we, wv in waits:
                    if wv > 0:
                        h.wait_ge(sems[we], wv)
                fn().then_inc(sems[sem], inc)
        for s, tot in totals.items():
            if tot > 0:
                nc.sync.wait_ge(sems[s], tot)

    return nc


def _prep_static(embed_table, W_ih_f, W_ih_b):
    """Host-side quantization + layout (shared across cores)."""
    table8 = (embed_table * SX).astype(ml_dtypes.float8_e4m3fn)
    table_u16 = np.ascontiguousarray(table8.view(np.uint16))  # [V, 912]

    wcat = np.zeros((2 * PAD_PAIRS, G), np.float32)
    wcat[:D_IN, :2048] = W_ih_f
    wcat[:D_IN, 2048:] = W_ih_b
    w8 = (wcat * SW).astype(ml_dtypes.float8_e4m3fn)
    # [f, g] -> [p, c, k, j, g'] with f = 2*(k*128+p)+j, g = c*256+g'
    w2 = np.ascontiguousarray(
        w8.reshape(PAD_PAIRS, 2, NCH, CHW)
        .reshape(KT2, 128, 2, NCH, CHW)
        .transpose(1, 3, 0, 2, 4))
    return table_u16, w2


def _run_device(ids_np, embed_table, W_ih_f, W_ih_b):
    from concourse.bass_utils import run_bass_kernel_spmd

    if "nc" not in _nc_cache:
        _nc_cache["nc"] = _build_nc()
    nc = _nc_cache["nc"]

    table_u16, w2 = _prep_static(embed_table, W_ih_f, W_ih_b)

    in_maps = []
    for c in range(N_CORES):
        ids_loc = ids_np[c * B_LOC:(c + 1) * B_LOC].reshape(ROWS)
        # token order REVERSED within each 128-tile: SwInterleave consumes
        # weight columns last-first, so psum partition m = real token m.
        ids_tr = np.ascontiguousarray(
            ids_loc.reshape(RT, 128)[:, ::-1].T.astype(np.int32))
        in_maps.append({"table": table_u16, "w2": w2, "ids_t": ids_tr})

    res = run_bass_kernel_spmd(nc, in_maps, core_ids=list(range(N_CORES)))
    global _last_exec_ns
    _last_exec_ns = res.exec_time_ns
    xw = np.stack([np.asarray(res.results[c]["xw_out"])
                   .astype(np.float32) for c in range(N_CORES)])
    return xw.reshape(B, T, G) * np.float32(1.0 / SO)


_last_exec_ns = None


def _sigmoid(x):
    return 1.0 / (1.0 + np.exp(-x))


def _lstm(xw, W_hh):
    # xw: [T, B, 4H] f32; returns hs [T, B, H]
    n_b = xw.shape[1]
    h = np.zeros((n_b, H), np.float32)
    c = np.zeros((n_b, H), np.float32)
    hs = np.empty((T, n_b, H), np.float32)
    for t in range(T):
        g = xw[t] + h @ W_hh
        i, f, gg, o = np.split(g, 4, axis=-1)
        c = _sigmoid(f) * c + _sigmoid(i) * np.tanh(gg)
        h = _sigmoid(o) * np.tanh(c)
        hs[t] = h
    return hs


def kernel(ids, tags, embed_table, W_ih_f, W_hh_f, b_f, W_ih_b, W_hh_b,
           b_b, W_tag, b_tag, transitions):
    ids = np.asarray(ids, np.int32)
    tags = np.asarray(tags, np.int32)
    embed_table = np.asarray(embed_table, np.float32)
    W_ih_f = np.asarray(W_ih_f, np.float32)
    W_hh_f = np.asarray(W_hh_f, np.float32)
    b_f = np.asarray(b_f, np.float32)
    W_ih_b = np.asarray(W_ih_b, np.float32)
    W_hh_b = np.asarray(W_hh_b, np.float32)
    b_b = np.asarray(b_b, np.float32)
    W_tag = np.asarray(W_tag, np.float32)
    b_tag = np.asarray(b_tag, np.float32)
    transitions = np.asarray(transitions, np.float32)

    xw = _run_device(ids, embed_table, W_ih_f, W_ih_b)  # [B, T, 4096] f32

    xw_f = np.transpose(xw[:, :, :2048], (1, 0, 2)) + b_f  # [T, B, 2048]
    xw_b = np.transpose(xw[:, :, 2048:], (1, 0, 2)) + b_b

    hf = _lstm(xw_f, W_hh_f)                        # [T, B, H]
    hb = _lstm(xw_b[::-1], W_hh_b)[::-1]

    hcat = np.concatenate([hf, hb], axis=-1)        # [T, B, 2H]
    feats = hcat.reshape(T * B, 2 * H) @ W_tag + b_tag
    feats = np.transpose(feats.reshape(T, B, K), (1, 0, 2))  # [B, T, K]

    # CRF forward (vectorized over batch)
    alpha = np.full((B, K), NEG, np.float32)
    alpha[:, START] = 0.0
    for t in range(T):
        scores = alpha[:, None, :] + transitions[None, :, :] + feats[:, t, :, None]
        m = scores.max(axis=2)
        alpha = m + np.log(np.sum(np.exp(scores - m[:, :, None]), axis=2))
    fin = alpha + transitions[STOP][None, :]
    mf = fin.max(axis=1)
    log_z = mf + np.log(np.sum(np.exp(fin - mf[:, None]), axis=1))

    prev = np.concatenate([np.full((B, 1), START, np.int32), tags], axis=1)
    nxt = np.concatenate([tags, np.full((B, 1), STOP, np.int32)], axis=1)
    gold = transitions[nxt, prev].sum(axis=1)
    gold += np.take_along_axis(
        feats, tags[:, :, None], axis=2
    )[:, :, 0].sum(axis=1)

    return (log_z - gold).astype(np.float32)
